# revision 39
# baseline (speedup 1.0000x reference)
"""DTW loss kernel for Trainium2 (Bass/Tile), 8-core data-parallel.

Math (per batch element):
  cost[i,j] = ||s1[i]||^2 + ||s2[j]||^2 - 2 s1[i].s2[j]         (GEMM form)
  DTW[i,j]  = cost[i,j] + min(DTW[i-1,j], DTW[i,j-1], DTW[i-1,j-1])
  loss      = mean_b DTW[L1-1, L2-1]

Device mapping (per core; 16 batch elems per core):
  Phase A (cost): all matmuls bf16 (dot K=128 + rank-2 norm augmentation
    K=2, PSUM-accumulated). Transposes packed 3-to-a-bank in PSUM, one
    copy-out per tensor (Act, bf16 cast, -2 scale folded into s1T). Squares
    on Pool. Cost rows cast to bf16 and shear-gathered by direct SBUF->SBUF
    DMA (no DRAM bounce). Ordering: all of s2's prep first, then per-batch
    s1 prep immediately followed by that batch's row-chunk-0 cost matmuls,
    so the DP wavefront starts as early as possible; chunks 1-2 follow and
    overlap the running DP.
  Phase B (DP): staggered column-block wavefront, stagger TWO: block c
    (partitions 32c+b) does row i at step s = i + 2c. ONE custom fused DVE
    instruction per step (hand-written uOp program DTW_FUSED_ANT):
        y = m + c[j]; m = min(p[j], y); out r[j] = min(p[j]+c[j], y)
    which realizes r[j] = min(p[j], p[j-1], r[j-1]) + c[j] with carried
    state m[j] = min(p[j], r[j]). The scan initial m0 = min(r[-1], p[-1])
    is computed INSIDE the op's seed uop from two per-partition scalars
    (s0 = neighbour's row-i last column, s1 = its row-(i-1) last column,
    the same halo ring tile one step older). Halo transport is three
    partition-shift-32 copies per step (>32-partition engine APs must
    start at partition 0, so they cannot merge), issued two steps ahead
    of consumption on Pool, and partly on Act once its phase-A tail has
    drained (HSPLIT) - all off the DVE critical chain.
"""

import os
import sys


def _ensure_path():
    try:
        import concourse  # noqa: F401
    except ImportError:
        for p in ("/opt/trn_rl_repo", "/root/.axon_site/_ro/trn_rl_repo"):
            if os.path.isdir(p) and p not in sys.path:
                sys.path.insert(0, p)


import numpy as np

BIG = 1.0e30
N_CORES = 8
STG = 2  # wavefront stagger between adjacent column blocks


# --------------------------------------------------------------------------
# Custom fused DVE op (hand-written uOp program, v3/TRN2)
# --------------------------------------------------------------------------
def _np_fused_reference(in0, in1, s0, s1, imm2):
    p = in0.astype(np.float32)
    c = in1.astype(np.float32)
    m = np.broadcast_to(np.asarray(s0, np.float32), (p.shape[0],)).copy()
    out = np.empty_like(p)
    for j in range(p.shape[1]):
        y = m + c[:, j]
        m = np.minimum(p[:, j], y)
        out[:, j] = np.minimum(p[:, j] + c[:, j], y)
    return out


def _register_fused_op():
    from concourse.dve_ops import (
        OPS,
        CUSTOM_DVE_SPECS,
        _CUSTOM_DVE_ROW_BASE,
        _SUB_OPCODE_FOR_NAME,
        DveOp,
    )
    from concourse.dve_spec import Spec, Src0, Src1, C0, minn
    from concourse.dve_uop import (
        AluInp,
        AluOp,
        DelayInp,
        DveOpSpec,
        InpSel,
        OutPath,
        OutSel,
        Trigger,
        UopConfig,
    )

    name = "DTW_FUSED_ANT"
    if name in _SUB_OPCODE_FOR_NAME:
        return next(op for op in OPS if op.name == name)

    EN = 1

    def _build_uops():
        # seed: blk1 A-flop <- min(C0, C1)  (scan initial m0 = min(r[-1], p[-1]))
        seed = UopConfig()
        seed.enable_input(InpSel.CONST_0, 3)  # lane3 -> PREV_DELAY_2 at blk0
        seed.enable_input(InpSel.CONST_1, 4)  # lane4 -> PREV_DELAY_3 at blk0
        seed.repeat_count = 1
        seed.trigger = (Trigger.COUNT, Trigger.NONE, Trigger.NONE)
        seed.next_uop = (1, 0, 0)
        d0 = seed.datapath_config[0]
        d0.delay[2] = DelayInp.PREV_DELAY
        d0.delay[3] = DelayInp.PREV_DELAY
        d0.delay_enable[2] = EN
        d0.delay_enable[3] = EN
        d1 = seed.datapath_config[1]
        d1.enable_alu(AluOp.MIN, AluInp.PREV_DELAY_2, AluInp.PREV_DELAY_3)
        d1.alu_out_a_enable = EN

        # bubble: one dead issue cycle so the A-flop settles
        bubble = UopConfig()
        bubble.repeat_count = 1
        bubble.trigger = (Trigger.COUNT, Trigger.NONE, Trigger.NONE)
        bubble.next_uop = (2, 0, 0)

        # steady: one element per visit, then bubble
        st = UopConfig()
        st.enable_input(InpSel.SRC_0, 1)  # p -> lane1 -> PREV_DELAY_0
        st.enable_input(InpSel.SRC_1, 2)  # c -> lane2 -> PREV_DELAY_1
        st.require_inp0 = EN
        st.require_inp1 = EN
        st.repeat_count = 1
        st.trigger = (Trigger.SRC_TENSOR_DONE, Trigger.COUNT, Trigger.NONE)
        st.next_uop = (0, 1, 0)
        st.enable_output(OutSel.ALU_OUT, OutPath.WR0_LO)
        b = st.datapath_config
        b[0].enable_alu(AluOp.ADD, AluInp.PREV_DELAY_1, AluInp.NEXT_ALU_OUT_A)
        b[0].delay[0] = DelayInp.PREV_DELAY
        b[0].delay[1] = DelayInp.PREV_DELAY
        b[0].delay_enable[0] = EN
        b[0].delay_enable[1] = EN
        b[1].enable_alu(AluOp.MIN, AluInp.PREV_ALU_OUT, AluInp.PREV_DELAY_0)
        b[1].alu_out_a_enable = EN
        b[1].delay[0] = DelayInp.PREV_DELAY
        b[1].delay[1] = DelayInp.PREV_DELAY
        b[1].delay[2] = DelayInp.PREV_ALU_OUT  # latch y into lane2
        b[1].delay_enable[0] = EN
        b[1].delay_enable[1] = EN
        b[1].delay_enable[2] = EN
        b[2].enable_alu(AluOp.ADD, AluInp.PREV_DELAY_0, AluInp.PREV_DELAY_1)
        b[2].delay[2] = DelayInp.PREV_DELAY
        b[2].delay_enable[2] = EN
        b[3].enable_alu(AluOp.MIN, AluInp.PREV_ALU_OUT, AluInp.PREV_DELAY_2)
        for k in range(4, 8):
            b[k].pass_through_alu()
        return [seed, bubble, st]

    class _HandDveOp(DveOp):
        def compile(self, ver):
            from concourse.dve_ops import _COMPILE_CACHE, get_dve_sub_opcode

            key = (self.name, ver)
            if (r := _COMPILE_CACHE.get(key)) is not None:
                return r
            assert ver == "v3", "hand uops authored for v3 (TRN2)"
            result = DveOpSpec(
                name=self.name,
                opcode=get_dve_sub_opcode(self.name),
                uops=_build_uops(),
                rd1_en=True,
            )
            _COMPILE_CACHE[key] = result
            return result

    spec = Spec(body=minn(Src0, C0) + Src1, reference=_np_fused_reference)
    op = _HandDveOp(name=name, spec=spec, subdim=False, uops_sha={})
    OPS.append(op)
    _SUB_OPCODE_FOR_NAME[name] = _CUSTOM_DVE_ROW_BASE + len(OPS) - 1
    CUSTOM_DVE_SPECS[name] = spec
    return op


# --------------------------------------------------------------------------
# Kernel build
# --------------------------------------------------------------------------
def build_nc(B, L, D, NB=4, sim_safe=False):
    _ensure_path()
    import concourse.bacc as bacc
    import concourse.tile as tile
    from concourse import mybir

    fused = _register_fused_op()

    f32 = mybir.dt.float32
    bf16 = mybir.dt.bfloat16
    Alu = mybir.AluOpType
    Act = mybir.ActivationFunctionType

    BS = L // NB
    NSTEP = L + STG * (NB - 1)
    RC = (L + 127) // 128
    RD = 4  # DP ring depth
    assert B == 16 and NB == 4 and D == 128 and L % 128 == 0

    nc = bacc.Bacc("TRN2", target_bir_lowering=False, debug=False)
    s1 = nc.dram_tensor("s1", [B, L, D], f32, kind="ExternalInput").ap()
    s2 = nc.dram_tensor("s2", [B, L, D], f32, kind="ExternalInput").ap()
    ident = nc.dram_tensor("ident", [128, 128], f32, kind="ExternalInput").ap()
    out = nc.dram_tensor("dtw", [B], f32, kind="ExternalOutput").ap()

    from contextlib import ExitStack

    NQ = (B + 2) // 3  # aug packs: 3 batches per [128,L] pair at bases 0/32/64

    with tile.TileContext(nc) as tc, ExitStack() as ctx:
        pool = ctx.enter_context(tc.tile_pool(name="persist", bufs=1))
        lpool = ctx.enter_context(tc.tile_pool(name="loads", bufs=3))
        sqpool = ctx.enter_context(tc.tile_pool(name="sq", bufs=3))
        cspool = ctx.enter_context(tc.tile_pool(name="costsb", bufs=3))
        tpool = ctx.enter_context(tc.tile_pool(name="tpsum", bufs=2, space="PSUM"))
        npool = ctx.enter_context(tc.tile_pool(name="npsum", bufs=1, space="PSUM"))
        cpool = ctx.enter_context(tc.tile_pool(name="cpsum", bufs=2, space="PSUM"))

        # --- persistent tiles ---
        sheared = pool.tile([128, NSTEP * BS], bf16, tag="sheared")
        Dring = [
            pool.tile([128, BS], f32, tag=f"D{k}", name=f"Dring{k}")
            for k in range(RD)
        ]
        HR = 4  # halo ring: Hr[k] = shifted last column of a DP row; the
        # r-halo of step s doubles as the p-halo of step s+1.
        Hring = [
            pool.tile([128, 1], f32, tag=f"H{k}", name=f"Hring{k}")
            for k in range(HR)
        ]
        identf = pool.tile([128, 128], f32, tag="identf")
        q_ones = pool.tile([128, 1], bf16, tag="qones")
        ones = pool.tile([128, 1], bf16, tag="ones")
        ones_row = pool.tile([1, L], bf16, tag="ones_row")
        s1T = pool.tile([128, B * L], bf16, tag="s1T")  # -2*s1^T per batch
        s2T = pool.tile([128, B * L], bf16, tag="s2T")
        lq = [
            pool.tile([128, L], bf16, tag=f"lq{q}", name=f"lq{q}") for q in range(NQ)
        ]
        rq = [
            pool.tile([128, L], bf16, tag=f"rq{q}", name=f"rq{q}") for q in range(NQ)
        ]

        nc.sync.dma_start(identf[:, :], ident)
        nc.gpsimd.memset(q_ones[:, :], 0.25)
        nc.gpsimd.memset(ones[:, :], 1.0)
        nc.gpsimd.memset(ones_row[:, :], 1.0)
        for k in range(RD):
            nc.gpsimd.memset(Dring[k][:, :], BIG)
        for k in range(HR):
            nc.gpsimd.memset(Hring[k][:, :], BIG)
        for q in range(NQ):
            for r in range(min(3, B - 3 * q)):
                nc.sync.dma_start(lq[q][32 * r + 1 : 32 * r + 2, :], ones_row[:, :])
                nc.gpsimd.memset(rq[q][32 * r : 32 * r + 1, :], 1.0)
        if sim_safe:
            nc.gpsimd.memset(sheared[:, :], 0.0)
        else:
            # ramp-up triangles only (consumed while block c waits for row 0)
            for c in range(1, NB):
                g = 32 * c
                nc.gpsimd.memset(sheared[g : g + B, 0 : STG * c * BS], 0.0)

        def load_transpose(src, dst_slice, scale, b):
            ld = lpool.tile([128, RC * D], f32, tag="ld", name="ld")
            # s2 loads on SP; s1 loads via Pool SWDGE (Act is the s1-pass
            # bottleneck, Pool has slack there)
            eng = nc.sync if scale == 1.0 else nc.gpsimd
            eng.dma_start(
                ld[:, :].rearrange("p (rc d) -> p rc d", rc=RC),
                src[b].rearrange("(rc p) d -> p rc d", p=128),
            )
            tp = tpool.tile([128, L], f32, tag="tp", name="tp")
            for rc in range(RC):
                nc.tensor.transpose(
                    tp[:, rc * 128 : (rc + 1) * 128],
                    ld[:, rc * D : (rc + 1) * D],
                    identf[:, :],
                )
            nc.scalar.activation(dst_slice, tp[:, :], Act.Copy, scale=scale)

        def cost_chunk(rc, b):
            q, r = b // 3, b % 3
            cp = cpool.tile([128, L], f32, tag="cp", name="cp")
            nc.tensor.matmul(
                cp[:, :],
                s1T[:, b * L + rc * 128 : b * L + (rc + 1) * 128],
                s2T[:, b * L : (b + 1) * L],
                start=True,
                stop=False,
            )
            nc.tensor.matmul(
                cp[:, :],
                lq[q][32 * r : 32 * r + 2, rc * 128 : (rc + 1) * 128],
                rq[q][32 * r : 32 * r + 2, :],
                start=False,
                stop=True,
            )
            csb = cspool.tile([128, L], bf16, tag="csb", name="csb")
            nc.scalar.activation(csb[:, :], cp[:, :], Act.Copy)
            for c in range(NB):
                # rc0 (the DP-start gate): Act is the s1-pass bottleneck,
                # so it gets only one of the four issues there.
                if rc == 0:
                    deng = nc.scalar if c == 3 else nc.sync
                else:
                    deng = nc.sync if c % 2 == 0 else nc.scalar
                deng.dma_start(
                    sheared[
                        32 * c + b : 32 * c + b + 1,
                        (rc * 128 + STG * c) * BS : (rc * 128 + 128 + STG * c) * BS,
                    ],
                    csb[:, c * BS : (c + 1) * BS],
                )

        # --- phase A, s2 pass: transposes + n2 for all batches ---
        for b in range(B):
            q, r = b // 3, b % 3
            sl2 = s2T[:, b * L : (b + 1) * L]
            load_transpose(s2, sl2, 1.0, b)
            sq2 = sqpool.tile([128, L], bf16, tag="sq2", name="sq2")
            nc.gpsimd.tensor_tensor(out=sq2[:, :], in0=sl2, in1=sl2, op=Alu.mult)
            n2p = npool.tile([1, L], f32, tag="n2p", name="n2p")
            nc.tensor.matmul(n2p[:, :], ones[:, :], sq2[:, :], start=True, stop=True)
            n2sb = sqpool.tile([1, L], bf16, tag="n2sb", name="n2sb")
            nc.scalar.activation(n2sb[:, :], n2p[:, :], Act.Copy)
            nc.sync.dma_start(rq[q][32 * r + 1 : 32 * r + 2, :], n2sb[:, :])

        # --- phase A, s1 pass + rc-0 cost (starts the wavefront early) ---
        for b in range(B):
            q, r = b // 3, b % 3
            sl1 = s1T[:, b * L : (b + 1) * L]
            load_transpose(s1, sl1, -2.0, b)
            sq1 = sqpool.tile([128, L], bf16, tag="sq1", name="sq1")
            nc.gpsimd.tensor_tensor(out=sq1[:, :], in0=sl1, in1=sl1, op=Alu.mult)
            n1p = npool.tile([1, L], f32, tag="n1p", name="n1p")
            nc.tensor.matmul(n1p[:, :], q_ones[:, :], sq1[:, :], start=True, stop=True)
            nc.vector.tensor_copy(lq[q][32 * r : 32 * r + 1, :], n1p[:, :])
            cost_chunk(0, b)

        # --- phase A tail: row chunks 1..RC-1 (overlap the running DP) ---
        for rc in range(1, RC):
            for b in range(B):
                cost_chunk(rc, b)

        # --- phase B: staggered wavefront DP, one fused DVE op per step ---
        # After Act's phase-A tail drains (~step 180 of the wavefront), move
        # two of the three halo copies to Act; before that, Act is still
        # busy with rc1/rc2 cost work and the offload backfires.
        HSPLIT = 130

        def emit_halo(s):
            # Hr_{s+2}[32c+b] <- r of step s, block c-1, last column. Serves
            # scan_{s+2} as r[-1] (s0) and scan_{s+3} as p[-1] (s1).
            Ha = Dring[s % RD]
            H = Hring[(s + 2) % HR]
            for g in range(1, NB):
                dst = H[32 * g : 32 * g + 32, 0:1]
                src = Ha[32 * (g - 1) : 32 * (g - 1) + 32, BS - 1 : BS]
                if s >= HSPLIT and g >= 2:
                    nc.scalar.activation(dst, src, Act.Copy)
                else:
                    nc.gpsimd.tensor_copy(dst, src)

        nc.vector._custom_dve(
            fused,
            out=Dring[0][0:B, :],
            in0=Dring[RD - 1][0:B, :],  # all BIG -> out = cumsum
            in1=sheared[0:B, 0:BS],
            s0=0.0,
            s1=0.0,
        )
        for s in range(1, NSTEP):
            nc.vector._custom_dve(
                fused,
                out=Dring[s % RD][:, :],
                in0=Dring[(s - 1) % RD][:, :],
                in1=sheared[:, s * BS : (s + 1) * BS],
                s0=Hring[s % HR][:, 0:1],
                s1=Hring[(s - 1) % HR][:, 0:1],
            )
            if s + 2 < NSTEP:
                emit_halo(s)

        Dlast = Dring[(NSTEP - 1) % RD]
        gl = 32 * (NB - 1)
        nc.sync.dma_start(out, Dlast[gl : gl + B, BS - 1 : BS])

    nc.finalize()
    return nc


_IDENT = np.eye(128, dtype=np.float32)


def kernel(s1_batch: np.ndarray, s2_batch: np.ndarray) -> np.ndarray:
    _ensure_path()
    from concourse.bass_utils import run_bass_kernel_spmd

    s1 = np.ascontiguousarray(s1_batch, dtype=np.float32)
    s2 = np.ascontiguousarray(s2_batch, dtype=np.float32)
    Btot, L, D = s1.shape
    assert Btot % N_CORES == 0
    B = Btot // N_CORES

    nc = build_nc(B, L, D, NB=4)
    in_maps = [
        {
            "s1": s1[i * B : (i + 1) * B],
            "s2": s2[i * B : (i + 1) * B],
            "ident": _IDENT,
        }
        for i in range(N_CORES)
    ]
    res = run_bass_kernel_spmd(nc, in_maps, list(range(N_CORES)))
    finals = np.concatenate([r["dtw"] for r in res.results])
    return np.array(np.mean(finals.astype(np.float64)), dtype=np.float32)


# revision 40
# speedup vs baseline: 1.0765x; 1.0765x over previous
"""DTW loss kernel for Trainium2 (Bass/Tile), 8-core data-parallel.

Math (per batch element):
  cost[i,j] = ||s1[i]||^2 + ||s2[j]||^2 - 2 s1[i].s2[j]         (GEMM form)
  DTW[i,j]  = cost[i,j] + min(DTW[i-1,j], DTW[i,j-1], DTW[i-1,j-1])
  loss      = mean_b DTW[L1-1, L2-1]

Device mapping (per core; 16 batch elems per core):
  Phase A (cost): all matmuls bf16 (dot K=128 + rank-2 norm augmentation
    K=2, PSUM-accumulated). Transposes packed 3-to-a-bank in PSUM, one
    copy-out per tensor (Act, bf16 cast, -2 scale folded into s1T). Squares
    on Pool. Cost rows cast to bf16 and shear-gathered by direct SBUF->SBUF
    DMA (no DRAM bounce). Ordering: all of s2's prep first, then per-batch
    s1 prep immediately followed by that batch's row-chunk-0 cost matmuls,
    so the DP wavefront starts as early as possible; chunks 1-2 follow and
    overlap the running DP.
  Phase B (DP): staggered column-block wavefront, stagger TWO: block c
    (partitions 32c+b) does row i at step s = i + 2c. ONE custom fused DVE
    instruction per step (hand-written uOp program DTW_FUSED_ANT):
        y = m + c[j]; m = min(p[j], y); out r[j] = min(p[j]+c[j], y)
    which realizes r[j] = min(p[j], p[j-1], r[j-1]) + c[j] with carried
    state m[j] = min(p[j], r[j]). The scan initial m0 = min(r[-1], p[-1])
    is computed INSIDE the op's seed uop from two per-partition scalars
    (s0 = neighbour's row-i last column, s1 = its row-(i-1) last column,
    the same halo ring tile one step older). Halo transport is three
    partition-shift-32 copies per step (>32-partition engine APs must
    start at partition 0, so they cannot merge), issued two steps ahead
    of consumption on Pool, and partly on Act once its phase-A tail has
    drained (HSPLIT) - all off the DVE critical chain.
"""

import os
import sys


def _ensure_path():
    try:
        import concourse  # noqa: F401
    except ImportError:
        for p in ("/opt/trn_rl_repo", "/root/.axon_site/_ro/trn_rl_repo"):
            if os.path.isdir(p) and p not in sys.path:
                sys.path.insert(0, p)


import numpy as np

BIG = 1.0e30
N_CORES = 8
STG = 2  # wavefront stagger between adjacent column blocks


# --------------------------------------------------------------------------
# Custom fused DVE op (hand-written uOp program, v3/TRN2)
# --------------------------------------------------------------------------
def _np_fused_reference(in0, in1, s0, s1, imm2):
    p = in0.astype(np.float32)
    c = in1.astype(np.float32)
    m = np.broadcast_to(np.asarray(s0, np.float32), (p.shape[0],)).copy()
    out = np.empty_like(p)
    for j in range(p.shape[1]):
        y = m + c[:, j]
        m = np.minimum(p[:, j], y)
        out[:, j] = np.minimum(p[:, j] + c[:, j], y)
    return out


def _register_fused_op():
    from concourse.dve_ops import (
        OPS,
        CUSTOM_DVE_SPECS,
        _CUSTOM_DVE_ROW_BASE,
        _SUB_OPCODE_FOR_NAME,
        DveOp,
    )
    from concourse.dve_spec import Spec, Src0, Src1, C0, minn
    from concourse.dve_uop import (
        AluInp,
        AluOp,
        DelayInp,
        DveOpSpec,
        InpSel,
        OutPath,
        OutSel,
        Trigger,
        UopConfig,
    )

    name = "DTW_FUSED_ANT"
    if name in _SUB_OPCODE_FOR_NAME:
        return next(op for op in OPS if op.name == name)

    EN = 1

    def _build_uops():
        # seed: blk1 A-flop <- min(C0, C1)  (scan initial m0 = min(r[-1], p[-1]))
        seed = UopConfig()
        seed.enable_input(InpSel.CONST_0, 3)  # lane3 -> PREV_DELAY_2 at blk0
        seed.enable_input(InpSel.CONST_1, 4)  # lane4 -> PREV_DELAY_3 at blk0
        seed.repeat_count = 1
        seed.trigger = (Trigger.COUNT, Trigger.NONE, Trigger.NONE)
        seed.next_uop = (1, 0, 0)
        d0 = seed.datapath_config[0]
        d0.delay[2] = DelayInp.PREV_DELAY
        d0.delay[3] = DelayInp.PREV_DELAY
        d0.delay_enable[2] = EN
        d0.delay_enable[3] = EN
        d1 = seed.datapath_config[1]
        d1.enable_alu(AluOp.MIN, AluInp.PREV_DELAY_2, AluInp.PREV_DELAY_3)
        d1.alu_out_a_enable = EN

        # bubble: one dead issue cycle so the A-flop settles
        bubble = UopConfig()
        bubble.repeat_count = 1
        bubble.trigger = (Trigger.COUNT, Trigger.NONE, Trigger.NONE)
        bubble.next_uop = (2, 0, 0)

        # steady: one element per visit, then bubble
        st = UopConfig()
        st.enable_input(InpSel.SRC_0, 1)  # p -> lane1 -> PREV_DELAY_0
        st.enable_input(InpSel.SRC_1, 2)  # c -> lane2 -> PREV_DELAY_1
        st.require_inp0 = EN
        st.require_inp1 = EN
        st.repeat_count = 1
        st.trigger = (Trigger.SRC_TENSOR_DONE, Trigger.COUNT, Trigger.NONE)
        st.next_uop = (0, 1, 0)
        st.enable_output(OutSel.ALU_OUT, OutPath.WR0_LO)
        b = st.datapath_config
        b[0].enable_alu(AluOp.ADD, AluInp.PREV_DELAY_1, AluInp.NEXT_ALU_OUT_A)
        b[0].delay[0] = DelayInp.PREV_DELAY
        b[0].delay[1] = DelayInp.PREV_DELAY
        b[0].delay_enable[0] = EN
        b[0].delay_enable[1] = EN
        b[1].enable_alu(AluOp.MIN, AluInp.PREV_ALU_OUT, AluInp.PREV_DELAY_0)
        b[1].alu_out_a_enable = EN
        b[1].delay[0] = DelayInp.PREV_DELAY
        b[1].delay[1] = DelayInp.PREV_DELAY
        b[1].delay[2] = DelayInp.PREV_ALU_OUT  # latch y into lane2
        b[1].delay_enable[0] = EN
        b[1].delay_enable[1] = EN
        b[1].delay_enable[2] = EN
        b[2].enable_alu(AluOp.ADD, AluInp.PREV_DELAY_0, AluInp.PREV_DELAY_1)
        b[2].delay[2] = DelayInp.PREV_DELAY
        b[2].delay_enable[2] = EN
        b[3].enable_alu(AluOp.MIN, AluInp.PREV_ALU_OUT, AluInp.PREV_DELAY_2)
        for k in range(4, 8):
            b[k].pass_through_alu()
        return [seed, bubble, st]

    class _HandDveOp(DveOp):
        def compile(self, ver):
            from concourse.dve_ops import _COMPILE_CACHE, get_dve_sub_opcode

            key = (self.name, ver)
            if (r := _COMPILE_CACHE.get(key)) is not None:
                return r
            assert ver == "v3", "hand uops authored for v3 (TRN2)"
            result = DveOpSpec(
                name=self.name,
                opcode=get_dve_sub_opcode(self.name),
                uops=_build_uops(),
                rd1_en=True,
            )
            _COMPILE_CACHE[key] = result
            return result

    spec = Spec(body=minn(Src0, C0) + Src1, reference=_np_fused_reference)
    op = _HandDveOp(name=name, spec=spec, subdim=False, uops_sha={})
    OPS.append(op)
    _SUB_OPCODE_FOR_NAME[name] = _CUSTOM_DVE_ROW_BASE + len(OPS) - 1
    CUSTOM_DVE_SPECS[name] = spec
    return op


# --------------------------------------------------------------------------
# Kernel build
# --------------------------------------------------------------------------
def build_nc(B, L, D, NB=4, sim_safe=False):
    _ensure_path()
    import concourse.bacc as bacc
    import concourse.tile as tile
    from concourse import mybir

    fused = _register_fused_op()

    f32 = mybir.dt.float32
    bf16 = mybir.dt.bfloat16
    Alu = mybir.AluOpType
    Act = mybir.ActivationFunctionType

    BS = L // NB
    NSTEP = L + STG * (NB - 1)
    RC = (L + 127) // 128
    RD = 4  # DP ring depth
    assert B == 16 and NB == 4 and D == 128 and L % 128 == 0

    nc = bacc.Bacc("TRN2", target_bir_lowering=False, debug=False)
    s1 = nc.dram_tensor("s1", [B, L, D], f32, kind="ExternalInput").ap()
    s2 = nc.dram_tensor("s2", [B, L, D], f32, kind="ExternalInput").ap()
    ident = nc.dram_tensor("ident", [128, 128], f32, kind="ExternalInput").ap()
    out = nc.dram_tensor("dtw", [B], f32, kind="ExternalOutput").ap()

    from contextlib import ExitStack

    NQ = (B + 2) // 3  # aug packs: 3 batches per [128,L] pair at bases 0/32/64

    with tile.TileContext(nc) as tc, ExitStack() as ctx:
        pool = ctx.enter_context(tc.tile_pool(name="persist", bufs=1))
        lpool = ctx.enter_context(tc.tile_pool(name="loads", bufs=4))
        sqpool = ctx.enter_context(tc.tile_pool(name="sq", bufs=3))
        cspool = ctx.enter_context(tc.tile_pool(name="costsb", bufs=4))
        tpool = ctx.enter_context(tc.tile_pool(name="tpsum", bufs=3, space="PSUM"))
        npool = ctx.enter_context(tc.tile_pool(name="npsum", bufs=1, space="PSUM"))
        cpool = ctx.enter_context(tc.tile_pool(name="cpsum", bufs=3, space="PSUM"))

        # --- persistent tiles ---
        sheared = pool.tile([128, NSTEP * BS], bf16, tag="sheared")
        Dring = [
            pool.tile([128, BS], f32, tag=f"D{k}", name=f"Dring{k}")
            for k in range(RD)
        ]
        HR = 4  # halo ring: Hr[k] = shifted last column of a DP row; the
        # r-halo of step s doubles as the p-halo of step s+1.
        Hring = [
            pool.tile([128, 1], f32, tag=f"H{k}", name=f"Hring{k}")
            for k in range(HR)
        ]
        identf = pool.tile([128, 128], f32, tag="identf")
        q_ones = pool.tile([128, 1], bf16, tag="qones")
        ones = pool.tile([128, 1], bf16, tag="ones")
        ones_row = pool.tile([1, L], bf16, tag="ones_row")
        s1T = pool.tile([128, B * L], bf16, tag="s1T")  # -2*s1^T per batch
        s2T = pool.tile([128, B * L], bf16, tag="s2T")
        lq = [
            pool.tile([128, L], bf16, tag=f"lq{q}", name=f"lq{q}") for q in range(NQ)
        ]
        rq = [
            pool.tile([128, L], bf16, tag=f"rq{q}", name=f"rq{q}") for q in range(NQ)
        ]

        nc.sync.dma_start(identf[:, :], ident)
        nc.gpsimd.memset(q_ones[:, :], 0.25)
        nc.gpsimd.memset(ones[:, :], 1.0)
        nc.gpsimd.memset(ones_row[:, :], 1.0)
        for k in range(RD):
            nc.gpsimd.memset(Dring[k][:, :], BIG)
        for k in range(HR):
            nc.gpsimd.memset(Hring[k][:, :], BIG)
        for q in range(NQ):
            for r in range(min(3, B - 3 * q)):
                nc.sync.dma_start(lq[q][32 * r + 1 : 32 * r + 2, :], ones_row[:, :])
                nc.gpsimd.memset(rq[q][32 * r : 32 * r + 1, :], 1.0)
        if sim_safe:
            nc.gpsimd.memset(sheared[:, :], 0.0)
        else:
            # ramp-up triangles only (consumed while block c waits for row 0)
            for c in range(1, NB):
                g = 32 * c
                nc.gpsimd.memset(sheared[g : g + B, 0 : STG * c * BS], 0.0)

        def load_transpose(src, dst_slice, scale, b):
            ld = lpool.tile([128, RC * D], f32, tag="ld", name="ld")
            # s2 loads on SP; s1 loads via Pool SWDGE (Act is the s1-pass
            # bottleneck, Pool has slack there)
            eng = nc.sync if scale == 1.0 else nc.gpsimd
            eng.dma_start(
                ld[:, :].rearrange("p (rc d) -> p rc d", rc=RC),
                src[b].rearrange("(rc p) d -> p rc d", p=128),
            )
            tp = tpool.tile([128, L], f32, tag="tp", name="tp")
            for rc in range(RC):
                nc.tensor.transpose(
                    tp[:, rc * 128 : (rc + 1) * 128],
                    ld[:, rc * D : (rc + 1) * D],
                    identf[:, :],
                )
            nc.scalar.activation(dst_slice, tp[:, :], Act.Copy, scale=scale)

        def cost_chunk(rc, b):
            q, r = b // 3, b % 3
            cp = cpool.tile([128, L], f32, tag="cp", name="cp")
            nc.tensor.matmul(
                cp[:, :],
                s1T[:, b * L + rc * 128 : b * L + (rc + 1) * 128],
                s2T[:, b * L : (b + 1) * L],
                start=True,
                stop=False,
            )
            nc.tensor.matmul(
                cp[:, :],
                lq[q][32 * r : 32 * r + 2, rc * 128 : (rc + 1) * 128],
                rq[q][32 * r : 32 * r + 2, :],
                start=False,
                stop=True,
            )
            csb = cspool.tile([128, L], bf16, tag="csb", name="csb")
            nc.scalar.activation(csb[:, :], cp[:, :], Act.Copy)
            for c in range(NB):
                deng = nc.sync if c % 2 == 0 else nc.scalar
                deng.dma_start(
                    sheared[
                        32 * c + b : 32 * c + b + 1,
                        (rc * 128 + STG * c) * BS : (rc * 128 + 128 + STG * c) * BS,
                    ],
                    csb[:, c * BS : (c + 1) * BS],
                )

        # --- phase A, s2 pass: transposes + n2 for all batches ---
        for b in range(B):
            q, r = b // 3, b % 3
            sl2 = s2T[:, b * L : (b + 1) * L]
            load_transpose(s2, sl2, 1.0, b)
            sq2 = sqpool.tile([128, L], bf16, tag="sq2", name="sq2")
            nc.gpsimd.tensor_tensor(out=sq2[:, :], in0=sl2, in1=sl2, op=Alu.mult)
            n2p = npool.tile([1, L], f32, tag="n2p", name="n2p")
            nc.tensor.matmul(n2p[:, :], ones[:, :], sq2[:, :], start=True, stop=True)
            n2sb = sqpool.tile([1, L], bf16, tag="n2sb", name="n2sb")
            nc.scalar.activation(n2sb[:, :], n2p[:, :], Act.Copy)
            nc.sync.dma_start(rq[q][32 * r + 1 : 32 * r + 2, :], n2sb[:, :])

        # --- phase A, s1 pass + rc-0 cost (starts the wavefront early) ---
        for b in range(B):
            q, r = b // 3, b % 3
            sl1 = s1T[:, b * L : (b + 1) * L]
            load_transpose(s1, sl1, -2.0, b)
            sq1 = sqpool.tile([128, L], bf16, tag="sq1", name="sq1")
            nc.gpsimd.tensor_tensor(out=sq1[:, :], in0=sl1, in1=sl1, op=Alu.mult)
            n1p = npool.tile([1, L], f32, tag="n1p", name="n1p")
            nc.tensor.matmul(n1p[:, :], q_ones[:, :], sq1[:, :], start=True, stop=True)
            nc.vector.tensor_copy(lq[q][32 * r : 32 * r + 1, :], n1p[:, :])
            cost_chunk(0, b)

        # --- phase A tail: row chunks 1..RC-1 (overlap the running DP) ---
        for rc in range(1, RC):
            for b in range(B):
                cost_chunk(rc, b)

        # --- phase B: staggered wavefront DP, one fused DVE op per step ---
        # After Act's phase-A tail drains (~step 180 of the wavefront), move
        # two of the three halo copies to Act; before that, Act is still
        # busy with rc1/rc2 cost work and the offload backfires.
        HSPLIT = 130

        def emit_halo(s):
            # Hr_{s+2}[32c+b] <- r of step s, block c-1, last column. Serves
            # scan_{s+2} as r[-1] (s0) and scan_{s+3} as p[-1] (s1).
            Ha = Dring[s % RD]
            H = Hring[(s + 2) % HR]
            for g in range(1, NB):
                dst = H[32 * g : 32 * g + 32, 0:1]
                src = Ha[32 * (g - 1) : 32 * (g - 1) + 32, BS - 1 : BS]
                if s >= HSPLIT and g >= 2:
                    nc.scalar.activation(dst, src, Act.Copy)
                else:
                    nc.gpsimd.tensor_copy(dst, src)

        nc.vector._custom_dve(
            fused,
            out=Dring[0][0:B, :],
            in0=Dring[RD - 1][0:B, :],  # all BIG -> out = cumsum
            in1=sheared[0:B, 0:BS],
            s0=0.0,
            s1=0.0,
        )
        for s in range(1, NSTEP):
            nc.vector._custom_dve(
                fused,
                out=Dring[s % RD][:, :],
                in0=Dring[(s - 1) % RD][:, :],
                in1=sheared[:, s * BS : (s + 1) * BS],
                s0=Hring[s % HR][:, 0:1],
                s1=Hring[(s - 1) % HR][:, 0:1],
            )
            if s + 2 < NSTEP:
                emit_halo(s)

        Dlast = Dring[(NSTEP - 1) % RD]
        gl = 32 * (NB - 1)
        nc.sync.dma_start(out, Dlast[gl : gl + B, BS - 1 : BS])

    nc.finalize()
    return nc


_IDENT = np.eye(128, dtype=np.float32)


def kernel(s1_batch: np.ndarray, s2_batch: np.ndarray) -> np.ndarray:
    _ensure_path()
    from concourse.bass_utils import run_bass_kernel_spmd

    s1 = np.ascontiguousarray(s1_batch, dtype=np.float32)
    s2 = np.ascontiguousarray(s2_batch, dtype=np.float32)
    Btot, L, D = s1.shape
    assert Btot % N_CORES == 0
    B = Btot // N_CORES

    nc = build_nc(B, L, D, NB=4)
    in_maps = [
        {
            "s1": s1[i * B : (i + 1) * B],
            "s2": s2[i * B : (i + 1) * B],
            "ident": _IDENT,
        }
        for i in range(N_CORES)
    ]
    res = run_bass_kernel_spmd(nc, in_maps, list(range(N_CORES)))
    finals = np.concatenate([r["dtw"] for r in res.results])
    return np.array(np.mean(finals.astype(np.float64)), dtype=np.float32)


# revision 41
# speedup vs baseline: 1.0922x; 1.0146x over previous
"""DTW loss kernel for Trainium2 (Bass/Tile), 8-core data-parallel.

Math (per batch element):
  cost[i,j] = ||s1[i]||^2 + ||s2[j]||^2 - 2 s1[i].s2[j]         (GEMM form)
  DTW[i,j]  = cost[i,j] + min(DTW[i-1,j], DTW[i,j-1], DTW[i-1,j-1])
  loss      = mean_b DTW[L1-1, L2-1]

Device mapping (per core; 16 batch elems per core):
  Phase A (cost): all matmuls bf16 (dot K=128 + rank-2 norm augmentation
    K=2, PSUM-accumulated). Transposes packed 3-to-a-bank in PSUM, one
    copy-out per tensor (Act, bf16 cast, -2 scale folded into s1T). Squares
    on Pool. Cost rows cast to bf16 and shear-gathered by direct SBUF->SBUF
    DMA (no DRAM bounce). Ordering: all of s2's prep first, then per-batch
    s1 prep immediately followed by that batch's row-chunk-0 cost matmuls,
    so the DP wavefront starts as early as possible; chunks 1-2 follow and
    overlap the running DP.
  Phase B (DP): staggered column-block wavefront, stagger TWO: block c
    (partitions 32c+b) does row i at step s = i + 2c. ONE custom fused DVE
    instruction per step (hand-written uOp program DTW_FUSED_ANT):
        y = m + c[j]; m = min(p[j], y); out r[j] = min(p[j]+c[j], y)
    which realizes r[j] = min(p[j], p[j-1], r[j-1]) + c[j] with carried
    state m[j] = min(p[j], r[j]). The scan initial m0 = min(r[-1], p[-1])
    is computed INSIDE the op's seed uop from two per-partition scalars
    (s0 = neighbour's row-i last column, s1 = its row-(i-1) last column,
    the same halo ring tile one step older). Halo transport is three
    partition-shift-32 copies per step (>32-partition engine APs must
    start at partition 0, so they cannot merge), issued two steps ahead
    of consumption on Pool, and partly on Act once its phase-A tail has
    drained (HSPLIT) - all off the DVE critical chain.
"""

import os
import sys


def _ensure_path():
    try:
        import concourse  # noqa: F401
    except ImportError:
        for p in ("/opt/trn_rl_repo", "/root/.axon_site/_ro/trn_rl_repo"):
            if os.path.isdir(p) and p not in sys.path:
                sys.path.insert(0, p)


import numpy as np

BIG = 1.0e30
N_CORES = 8
STG = 2  # wavefront stagger between adjacent column blocks


# --------------------------------------------------------------------------
# Custom fused DVE op (hand-written uOp program, v3/TRN2)
# --------------------------------------------------------------------------
def _np_fused_reference(in0, in1, s0, s1, imm2):
    p = in0.astype(np.float32)
    c = in1.astype(np.float32)
    m = np.broadcast_to(np.asarray(s0, np.float32), (p.shape[0],)).copy()
    out = np.empty_like(p)
    for j in range(p.shape[1]):
        y = m + c[:, j]
        m = np.minimum(p[:, j], y)
        out[:, j] = np.minimum(p[:, j] + c[:, j], y)
    return out


def _register_fused_op():
    from concourse.dve_ops import (
        OPS,
        CUSTOM_DVE_SPECS,
        _CUSTOM_DVE_ROW_BASE,
        _SUB_OPCODE_FOR_NAME,
        DveOp,
    )
    from concourse.dve_spec import Spec, Src0, Src1, C0, minn
    from concourse.dve_uop import (
        AluInp,
        AluOp,
        DelayInp,
        DveOpSpec,
        InpSel,
        OutPath,
        OutSel,
        Trigger,
        UopConfig,
    )

    name = "DTW_FUSED_ANT"
    if name in _SUB_OPCODE_FOR_NAME:
        return next(op for op in OPS if op.name == name)

    EN = 1

    def _build_uops():
        # seed: blk1 A-flop <- min(C0, C1)  (scan initial m0 = min(r[-1], p[-1]))
        seed = UopConfig()
        seed.enable_input(InpSel.CONST_0, 3)  # lane3 -> PREV_DELAY_2 at blk0
        seed.enable_input(InpSel.CONST_1, 4)  # lane4 -> PREV_DELAY_3 at blk0
        seed.repeat_count = 1
        seed.trigger = (Trigger.COUNT, Trigger.NONE, Trigger.NONE)
        seed.next_uop = (1, 0, 0)
        d0 = seed.datapath_config[0]
        d0.delay[2] = DelayInp.PREV_DELAY
        d0.delay[3] = DelayInp.PREV_DELAY
        d0.delay_enable[2] = EN
        d0.delay_enable[3] = EN
        d1 = seed.datapath_config[1]
        d1.enable_alu(AluOp.MIN, AluInp.PREV_DELAY_2, AluInp.PREV_DELAY_3)
        d1.alu_out_a_enable = EN

        # bubble: one dead issue cycle so the A-flop settles
        bubble = UopConfig()
        bubble.repeat_count = 1
        bubble.trigger = (Trigger.COUNT, Trigger.NONE, Trigger.NONE)
        bubble.next_uop = (2, 0, 0)

        # steady: one element per visit, then bubble
        st = UopConfig()
        st.enable_input(InpSel.SRC_0, 1)  # p -> lane1 -> PREV_DELAY_0
        st.enable_input(InpSel.SRC_1, 2)  # c -> lane2 -> PREV_DELAY_1
        st.require_inp0 = EN
        st.require_inp1 = EN
        st.repeat_count = 1
        st.trigger = (Trigger.SRC_TENSOR_DONE, Trigger.COUNT, Trigger.NONE)
        st.next_uop = (0, 1, 0)
        st.enable_output(OutSel.ALU_OUT, OutPath.WR0_LO)
        b = st.datapath_config
        b[0].enable_alu(AluOp.ADD, AluInp.PREV_DELAY_1, AluInp.NEXT_ALU_OUT_A)
        b[0].delay[0] = DelayInp.PREV_DELAY
        b[0].delay[1] = DelayInp.PREV_DELAY
        b[0].delay_enable[0] = EN
        b[0].delay_enable[1] = EN
        b[1].enable_alu(AluOp.MIN, AluInp.PREV_ALU_OUT, AluInp.PREV_DELAY_0)
        b[1].alu_out_a_enable = EN
        b[1].delay[0] = DelayInp.PREV_DELAY
        b[1].delay[1] = DelayInp.PREV_DELAY
        b[1].delay[2] = DelayInp.PREV_ALU_OUT  # latch y into lane2
        b[1].delay_enable[0] = EN
        b[1].delay_enable[1] = EN
        b[1].delay_enable[2] = EN
        b[2].enable_alu(AluOp.ADD, AluInp.PREV_DELAY_0, AluInp.PREV_DELAY_1)
        b[2].delay[2] = DelayInp.PREV_DELAY
        b[2].delay_enable[2] = EN
        b[3].enable_alu(AluOp.MIN, AluInp.PREV_ALU_OUT, AluInp.PREV_DELAY_2)
        for k in range(4, 8):
            b[k].pass_through_alu()
        return [seed, bubble, st]

    class _HandDveOp(DveOp):
        def compile(self, ver):
            from concourse.dve_ops import _COMPILE_CACHE, get_dve_sub_opcode

            key = (self.name, ver)
            if (r := _COMPILE_CACHE.get(key)) is not None:
                return r
            assert ver == "v3", "hand uops authored for v3 (TRN2)"
            result = DveOpSpec(
                name=self.name,
                opcode=get_dve_sub_opcode(self.name),
                uops=_build_uops(),
                rd1_en=True,
            )
            _COMPILE_CACHE[key] = result
            return result

    spec = Spec(body=minn(Src0, C0) + Src1, reference=_np_fused_reference)
    op = _HandDveOp(name=name, spec=spec, subdim=False, uops_sha={})
    OPS.append(op)
    _SUB_OPCODE_FOR_NAME[name] = _CUSTOM_DVE_ROW_BASE + len(OPS) - 1
    CUSTOM_DVE_SPECS[name] = spec
    return op


# --------------------------------------------------------------------------
# Kernel build
# --------------------------------------------------------------------------
def build_nc(B, L, D, NB=4, sim_safe=False):
    _ensure_path()
    import concourse.bacc as bacc
    import concourse.tile as tile
    from concourse import mybir

    fused = _register_fused_op()

    f32 = mybir.dt.float32
    bf16 = mybir.dt.bfloat16
    Alu = mybir.AluOpType
    Act = mybir.ActivationFunctionType

    BS = L // NB
    NSTEP = L + STG * (NB - 1)
    RC = (L + 127) // 128
    RD = 4  # DP ring depth
    assert B == 16 and NB == 4 and D == 128 and L % 128 == 0

    nc = bacc.Bacc("TRN2", target_bir_lowering=False, debug=False)
    s1 = nc.dram_tensor("s1", [B, L, D], f32, kind="ExternalInput").ap()
    s2 = nc.dram_tensor("s2", [B, L, D], f32, kind="ExternalInput").ap()
    ident = nc.dram_tensor("ident", [128, 128], f32, kind="ExternalInput").ap()
    out = nc.dram_tensor("dtw", [B], f32, kind="ExternalOutput").ap()

    from contextlib import ExitStack

    NQ = (B + 2) // 3  # aug packs: 3 batches per [128,L] pair at bases 0/32/64

    with tile.TileContext(nc) as tc, ExitStack() as ctx:
        pool = ctx.enter_context(tc.tile_pool(name="persist", bufs=1))
        lpool = ctx.enter_context(tc.tile_pool(name="loads", bufs=3))
        sqpool = ctx.enter_context(tc.tile_pool(name="sq", bufs=3))
        cspool = ctx.enter_context(tc.tile_pool(name="costsb", bufs=3))
        tpool = ctx.enter_context(tc.tile_pool(name="tpsum", bufs=2, space="PSUM"))
        npool = ctx.enter_context(tc.tile_pool(name="npsum", bufs=1, space="PSUM"))
        cpool = ctx.enter_context(tc.tile_pool(name="cpsum", bufs=2, space="PSUM"))

        # --- persistent tiles ---
        sheared = pool.tile([128, NSTEP * BS], bf16, tag="sheared")
        Dring = [
            pool.tile([128, BS], f32, tag=f"D{k}", name=f"Dring{k}")
            for k in range(RD)
        ]
        HR = 4  # halo ring: Hr[k] = shifted last column of a DP row; the
        # r-halo of step s doubles as the p-halo of step s+1.
        Hring = [
            pool.tile([128, 1], f32, tag=f"H{k}", name=f"Hring{k}")
            for k in range(HR)
        ]
        identf = pool.tile([128, 128], f32, tag="identf")
        q_ones = pool.tile([128, 1], bf16, tag="qones")
        ones = pool.tile([128, 1], bf16, tag="ones")
        ones_row = pool.tile([1, L], bf16, tag="ones_row")
        s1T = pool.tile([128, B * L], bf16, tag="s1T")  # -2*s1^T per batch
        s2T = pool.tile([128, B * L], bf16, tag="s2T")
        lq = [
            pool.tile([128, L], bf16, tag=f"lq{q}", name=f"lq{q}") for q in range(NQ)
        ]
        rq = [
            pool.tile([128, L], bf16, tag=f"rq{q}", name=f"rq{q}") for q in range(NQ)
        ]

        nc.sync.dma_start(identf[:, :], ident)
        nc.gpsimd.memset(q_ones[:, :], 0.25)
        nc.gpsimd.memset(ones[:, :], 1.0)
        nc.gpsimd.memset(ones_row[:, :], 1.0)
        for k in range(RD):
            nc.gpsimd.memset(Dring[k][:, :], BIG)
        for k in range(HR):
            nc.gpsimd.memset(Hring[k][:, :], BIG)
        for q in range(NQ):
            for r in range(min(3, B - 3 * q)):
                nc.sync.dma_start(lq[q][32 * r + 1 : 32 * r + 2, :], ones_row[:, :])
                nc.gpsimd.memset(rq[q][32 * r : 32 * r + 1, :], 1.0)
        if sim_safe:
            nc.gpsimd.memset(sheared[:, :], 0.0)
        else:
            # ramp-up triangles only (consumed while block c waits for row 0)
            for c in range(1, NB):
                g = 32 * c
                nc.gpsimd.memset(sheared[g : g + B, 0 : STG * c * BS], 0.0)

        def load_transpose(src, dst_slice, scale, b):
            ld = lpool.tile([128, RC * D], f32, tag="ld", name="ld")
            # s2 loads on SP; s1 loads via Pool SWDGE (Act is the s1-pass
            # bottleneck, Pool has slack there)
            eng = nc.sync if scale == 1.0 else nc.gpsimd
            eng.dma_start(
                ld[:, :].rearrange("p (rc d) -> p rc d", rc=RC),
                src[b].rearrange("(rc p) d -> p rc d", p=128),
            )
            tp = tpool.tile([128, L], f32, tag="tp", name="tp")
            for rc in range(RC):
                nc.tensor.transpose(
                    tp[:, rc * 128 : (rc + 1) * 128],
                    ld[:, rc * D : (rc + 1) * D],
                    identf[:, :],
                )
            nc.scalar.activation(dst_slice, tp[:, :], Act.Copy, scale=scale)

        def cost_chunk(rc, b):
            q, r = b // 3, b % 3
            cp = cpool.tile([128, L], f32, tag="cp", name="cp")
            nc.tensor.matmul(
                cp[:, :],
                s1T[:, b * L + rc * 128 : b * L + (rc + 1) * 128],
                s2T[:, b * L : (b + 1) * L],
                start=True,
                stop=False,
            )
            nc.tensor.matmul(
                cp[:, :],
                lq[q][32 * r : 32 * r + 2, rc * 128 : (rc + 1) * 128],
                rq[q][32 * r : 32 * r + 2, :],
                start=False,
                stop=True,
            )
            csb = cspool.tile([128, L], bf16, tag="csb", name="csb")
            nc.scalar.activation(csb[:, :], cp[:, :], Act.Copy)
            for c in range(NB):
                deng = nc.sync if c % 2 == 0 else nc.scalar
                deng.dma_start(
                    sheared[
                        32 * c + b : 32 * c + b + 1,
                        (rc * 128 + STG * c) * BS : (rc * 128 + 128 + STG * c) * BS,
                    ],
                    csb[:, c * BS : (c + 1) * BS],
                )

        # --- phase A, s2 pass: transposes + n2 for all batches ---
        for b in range(B):
            q, r = b // 3, b % 3
            sl2 = s2T[:, b * L : (b + 1) * L]
            load_transpose(s2, sl2, 1.0, b)
            sq2 = sqpool.tile([128, L], bf16, tag="sq2", name="sq2")
            nc.gpsimd.tensor_tensor(out=sq2[:, :], in0=sl2, in1=sl2, op=Alu.mult)
            n2p = npool.tile([1, L], f32, tag="n2p", name="n2p")
            nc.tensor.matmul(n2p[:, :], ones[:, :], sq2[:, :], start=True, stop=True)
            n2sb = sqpool.tile([1, L], bf16, tag="n2sb", name="n2sb")
            nc.scalar.activation(n2sb[:, :], n2p[:, :], Act.Copy)
            nc.sync.dma_start(rq[q][32 * r + 1 : 32 * r + 2, :], n2sb[:, :])

        # --- phase A, s1 pass + rc-0 cost (starts the wavefront early) ---
        for b in range(B):
            q, r = b // 3, b % 3
            sl1 = s1T[:, b * L : (b + 1) * L]
            load_transpose(s1, sl1, -2.0, b)
            sq1 = sqpool.tile([128, L], bf16, tag="sq1", name="sq1")
            nc.gpsimd.tensor_tensor(out=sq1[:, :], in0=sl1, in1=sl1, op=Alu.mult)
            n1p = npool.tile([1, L], f32, tag="n1p", name="n1p")
            nc.tensor.matmul(n1p[:, :], q_ones[:, :], sq1[:, :], start=True, stop=True)
            nc.vector.tensor_copy(lq[q][32 * r : 32 * r + 1, :], n1p[:, :])
            cost_chunk(0, b)

        # --- phase A tail: row chunks 1..RC-1 (overlap the running DP) ---
        for rc in range(1, RC):
            for b in range(B):
                cost_chunk(rc, b)

        # --- phase B: staggered wavefront DP, one fused DVE op per step ---
        # After Act's phase-A tail drains (~step 180 of the wavefront), move
        # two of the three halo copies to Act; before that, Act is still
        # busy with rc1/rc2 cost work and the offload backfires.
        HSPLIT = 130

        def emit_halo(s):
            # Hr_{s+2}[32c+b] <- r of step s, block c-1, last column. Serves
            # scan_{s+2} as r[-1] (s0) and scan_{s+3} as p[-1] (s1).
            Ha = Dring[s % RD]
            H = Hring[(s + 2) % HR]
            for g in range(1, NB):
                dst = H[32 * g : 32 * g + 32, 0:1]
                src = Ha[32 * (g - 1) : 32 * (g - 1) + 32, BS - 1 : BS]
                if s >= HSPLIT and g >= 2:
                    nc.scalar.activation(dst, src, Act.Copy)
                else:
                    nc.gpsimd.tensor_copy(dst, src)

        nc.vector._custom_dve(
            fused,
            out=Dring[0][0:B, :],
            in0=Dring[RD - 1][0:B, :],  # all BIG -> out = cumsum
            in1=sheared[0:B, 0:BS],
            s0=0.0,
            s1=0.0,
        )
        for s in range(1, NSTEP):
            nc.vector._custom_dve(
                fused,
                out=Dring[s % RD][:, :],
                in0=Dring[(s - 1) % RD][:, :],
                in1=sheared[:, s * BS : (s + 1) * BS],
                s0=Hring[s % HR][:, 0:1],
                s1=Hring[(s - 1) % HR][:, 0:1],
            )
            if s + 2 < NSTEP:
                emit_halo(s)

        Dlast = Dring[(NSTEP - 1) % RD]
        gl = 32 * (NB - 1)
        nc.sync.dma_start(out, Dlast[gl : gl + B, BS - 1 : BS])

    nc.finalize()
    return nc


_IDENT = np.eye(128, dtype=np.float32)


def kernel(s1_batch: np.ndarray, s2_batch: np.ndarray) -> np.ndarray:
    _ensure_path()
    from concourse.bass_utils import run_bass_kernel_spmd

    s1 = np.ascontiguousarray(s1_batch, dtype=np.float32)
    s2 = np.ascontiguousarray(s2_batch, dtype=np.float32)
    Btot, L, D = s1.shape
    assert Btot % N_CORES == 0
    B = Btot // N_CORES

    nc = build_nc(B, L, D, NB=4)
    in_maps = [
        {
            "s1": s1[i * B : (i + 1) * B],
            "s2": s2[i * B : (i + 1) * B],
            "ident": _IDENT,
        }
        for i in range(N_CORES)
    ]
    res = run_bass_kernel_spmd(nc, in_maps, list(range(N_CORES)))
    finals = np.concatenate([r["dtw"] for r in res.results])
    return np.array(np.mean(finals.astype(np.float64)), dtype=np.float32)


# revision 42
# speedup vs baseline: 1.0934x; 1.0011x over previous
"""DTW loss kernel for Trainium2 (Bass/Tile), 8-core data-parallel.

Math (per batch element):
  cost[i,j] = ||s1[i]||^2 + ||s2[j]||^2 - 2 s1[i].s2[j]         (GEMM form)
  DTW[i,j]  = cost[i,j] + min(DTW[i-1,j], DTW[i,j-1], DTW[i-1,j-1])
  loss      = mean_b DTW[L1-1, L2-1]

Device mapping (per core; 16 batch elems per core):
  Phase A (cost): all matmuls bf16 (dot K=128 + rank-2 norm augmentation
    K=2, PSUM-accumulated). Transposes packed 3-to-a-bank in PSUM, one
    copy-out per tensor (Act, bf16 cast, -2 scale folded into s1T). Squares
    on Pool. Cost rows cast to bf16 and shear-gathered by direct SBUF->SBUF
    DMA (no DRAM bounce). Ordering: all of s2's prep first, then per-batch
    s1 prep immediately followed by that batch's row-chunk-0 cost matmuls,
    so the DP wavefront starts as early as possible; chunks 1-2 follow and
    overlap the running DP.
  Phase B (DP): staggered column-block wavefront, stagger TWO: block c
    (partitions 32c+b) does row i at step s = i + 2c. ONE custom fused DVE
    instruction per step (hand-written uOp program DTW_FUSED_ANT):
        y = m + c[j]; m = min(p[j], y); out r[j] = min(p[j]+c[j], y)
    which realizes r[j] = min(p[j], p[j-1], r[j-1]) + c[j] with carried
    state m[j] = min(p[j], r[j]). The scan initial m0 = min(r[-1], p[-1])
    is computed INSIDE the op's seed uop from two per-partition scalars
    (s0 = neighbour's row-i last column, s1 = its row-(i-1) last column,
    the same halo ring tile one step older). Halo transport is three
    partition-shift-32 copies per step (>32-partition engine APs must
    start at partition 0, so they cannot merge), issued two steps ahead
    of consumption on Pool, and partly on Act once its phase-A tail has
    drained (HSPLIT) - all off the DVE critical chain.
"""

import os
import sys


def _ensure_path():
    try:
        import concourse  # noqa: F401
    except ImportError:
        for p in ("/opt/trn_rl_repo", "/root/.axon_site/_ro/trn_rl_repo"):
            if os.path.isdir(p) and p not in sys.path:
                sys.path.insert(0, p)


import numpy as np

BIG = 1.0e30
N_CORES = 8
STG = 2  # wavefront stagger between adjacent column blocks


# --------------------------------------------------------------------------
# Custom fused DVE op (hand-written uOp program, v3/TRN2)
# --------------------------------------------------------------------------
def _np_fused_reference(in0, in1, s0, s1, imm2):
    p = in0.astype(np.float32)
    c = in1.astype(np.float32)
    m = np.broadcast_to(np.asarray(s0, np.float32), (p.shape[0],)).copy()
    out = np.empty_like(p)
    for j in range(p.shape[1]):
        y = m + c[:, j]
        m = np.minimum(p[:, j], y)
        out[:, j] = np.minimum(p[:, j] + c[:, j], y)
    return out


def _register_fused_op():
    from concourse.dve_ops import (
        OPS,
        CUSTOM_DVE_SPECS,
        _CUSTOM_DVE_ROW_BASE,
        _SUB_OPCODE_FOR_NAME,
        DveOp,
    )
    from concourse.dve_spec import Spec, Src0, Src1, C0, minn
    from concourse.dve_uop import (
        AluInp,
        AluOp,
        DelayInp,
        DveOpSpec,
        InpSel,
        OutPath,
        OutSel,
        Trigger,
        UopConfig,
    )

    name = "DTW_FUSED_ANT"
    if name in _SUB_OPCODE_FOR_NAME:
        return next(op for op in OPS if op.name == name)

    EN = 1

    def _build_uops():
        # seed: blk1 A-flop <- min(C0, C1)  (scan initial m0 = min(r[-1], p[-1]))
        seed = UopConfig()
        seed.enable_input(InpSel.CONST_0, 3)  # lane3 -> PREV_DELAY_2 at blk0
        seed.enable_input(InpSel.CONST_1, 4)  # lane4 -> PREV_DELAY_3 at blk0
        seed.repeat_count = 1
        seed.trigger = (Trigger.COUNT, Trigger.NONE, Trigger.NONE)
        seed.next_uop = (1, 0, 0)
        d0 = seed.datapath_config[0]
        d0.delay[2] = DelayInp.PREV_DELAY
        d0.delay[3] = DelayInp.PREV_DELAY
        d0.delay_enable[2] = EN
        d0.delay_enable[3] = EN
        d1 = seed.datapath_config[1]
        d1.enable_alu(AluOp.MIN, AluInp.PREV_DELAY_2, AluInp.PREV_DELAY_3)
        d1.alu_out_a_enable = EN

        # bubble: one dead issue cycle so the A-flop settles
        bubble = UopConfig()
        bubble.repeat_count = 1
        bubble.trigger = (Trigger.COUNT, Trigger.NONE, Trigger.NONE)
        bubble.next_uop = (2, 0, 0)

        # steady: one element per visit, then bubble
        st = UopConfig()
        st.enable_input(InpSel.SRC_0, 1)  # p -> lane1 -> PREV_DELAY_0
        st.enable_input(InpSel.SRC_1, 2)  # c -> lane2 -> PREV_DELAY_1
        st.require_inp0 = EN
        st.require_inp1 = EN
        st.repeat_count = 1
        st.trigger = (Trigger.SRC_TENSOR_DONE, Trigger.COUNT, Trigger.NONE)
        st.next_uop = (0, 1, 0)
        st.enable_output(OutSel.ALU_OUT, OutPath.WR0_LO)
        b = st.datapath_config
        b[0].enable_alu(AluOp.ADD, AluInp.PREV_DELAY_1, AluInp.NEXT_ALU_OUT_A)
        b[0].delay[0] = DelayInp.PREV_DELAY
        b[0].delay[1] = DelayInp.PREV_DELAY
        b[0].delay_enable[0] = EN
        b[0].delay_enable[1] = EN
        b[1].enable_alu(AluOp.MIN, AluInp.PREV_ALU_OUT, AluInp.PREV_DELAY_0)
        b[1].alu_out_a_enable = EN
        b[1].delay[0] = DelayInp.PREV_DELAY
        b[1].delay[1] = DelayInp.PREV_DELAY
        b[1].delay[2] = DelayInp.PREV_ALU_OUT  # latch y into lane2
        b[1].delay_enable[0] = EN
        b[1].delay_enable[1] = EN
        b[1].delay_enable[2] = EN
        b[2].enable_alu(AluOp.ADD, AluInp.PREV_DELAY_0, AluInp.PREV_DELAY_1)
        b[2].delay[2] = DelayInp.PREV_DELAY
        b[2].delay_enable[2] = EN
        b[3].enable_alu(AluOp.MIN, AluInp.PREV_ALU_OUT, AluInp.PREV_DELAY_2)
        for k in range(4, 8):
            b[k].pass_through_alu()
        return [seed, bubble, st]

    class _HandDveOp(DveOp):
        def compile(self, ver):
            from concourse.dve_ops import _COMPILE_CACHE, get_dve_sub_opcode

            key = (self.name, ver)
            if (r := _COMPILE_CACHE.get(key)) is not None:
                return r
            assert ver == "v3", "hand uops authored for v3 (TRN2)"
            result = DveOpSpec(
                name=self.name,
                opcode=get_dve_sub_opcode(self.name),
                uops=_build_uops(),
                rd1_en=True,
            )
            _COMPILE_CACHE[key] = result
            return result

    spec = Spec(body=minn(Src0, C0) + Src1, reference=_np_fused_reference)
    op = _HandDveOp(name=name, spec=spec, subdim=False, uops_sha={})
    OPS.append(op)
    _SUB_OPCODE_FOR_NAME[name] = _CUSTOM_DVE_ROW_BASE + len(OPS) - 1
    CUSTOM_DVE_SPECS[name] = spec
    return op


# --------------------------------------------------------------------------
# Kernel build
# --------------------------------------------------------------------------
def build_nc(B, L, D, NB=4, sim_safe=False):
    _ensure_path()
    import concourse.bacc as bacc
    import concourse.tile as tile
    from concourse import mybir

    fused = _register_fused_op()

    f32 = mybir.dt.float32
    bf16 = mybir.dt.bfloat16
    Alu = mybir.AluOpType
    Act = mybir.ActivationFunctionType

    BS = L // NB
    NSTEP = L + STG * (NB - 1)
    RC = (L + 127) // 128
    RD = 4  # DP ring depth
    assert B == 16 and NB == 4 and D == 128 and L % 128 == 0

    nc = bacc.Bacc("TRN2", target_bir_lowering=False, debug=False)
    s1 = nc.dram_tensor("s1", [B, L, D], f32, kind="ExternalInput").ap()
    s2 = nc.dram_tensor("s2", [B, L, D], f32, kind="ExternalInput").ap()
    ident = nc.dram_tensor("ident", [128, 128], f32, kind="ExternalInput").ap()
    out = nc.dram_tensor("dtw", [B], f32, kind="ExternalOutput").ap()

    from contextlib import ExitStack

    NQ = (B + 2) // 3  # aug packs: 3 batches per [128,L] pair at bases 0/32/64

    with tile.TileContext(nc) as tc, ExitStack() as ctx:
        pool = ctx.enter_context(tc.tile_pool(name="persist", bufs=1))
        lpool = ctx.enter_context(tc.tile_pool(name="loads", bufs=3))
        sqpool = ctx.enter_context(tc.tile_pool(name="sq", bufs=3))
        cspool = ctx.enter_context(tc.tile_pool(name="costsb", bufs=3))
        tpool = ctx.enter_context(tc.tile_pool(name="tpsum", bufs=2, space="PSUM"))
        npool = ctx.enter_context(tc.tile_pool(name="npsum", bufs=1, space="PSUM"))
        cpool = ctx.enter_context(tc.tile_pool(name="cpsum", bufs=2, space="PSUM"))

        # --- persistent tiles ---
        sheared = pool.tile([128, NSTEP * BS], bf16, tag="sheared")
        Dring = [
            pool.tile([128, BS], f32, tag=f"D{k}", name=f"Dring{k}")
            for k in range(RD)
        ]
        # Flat halo array, one column per step (written once, never
        # rewritten -> no write-after-read edges back to the DVE). Column s
        # holds the r-halo consumed by scan_s as s0 and by scan_{s+1} as s1.
        Hall = pool.tile([128, NSTEP], f32, tag="Hall")
        identf = pool.tile([128, 128], f32, tag="identf")
        q_ones = pool.tile([128, 1], bf16, tag="qones")
        ones = pool.tile([128, 1], bf16, tag="ones")
        ones_row = pool.tile([1, L], bf16, tag="ones_row")
        s1T = pool.tile([128, B * L], bf16, tag="s1T")  # -2*s1^T per batch
        s2T = pool.tile([128, B * L], bf16, tag="s2T")
        lq = [
            pool.tile([128, L], bf16, tag=f"lq{q}", name=f"lq{q}") for q in range(NQ)
        ]
        rq = [
            pool.tile([128, L], bf16, tag=f"rq{q}", name=f"rq{q}") for q in range(NQ)
        ]

        nc.sync.dma_start(identf[:, :], ident)
        nc.gpsimd.memset(q_ones[:, :], 0.25)
        nc.gpsimd.memset(ones[:, :], 1.0)
        nc.gpsimd.memset(ones_row[:, :], 1.0)
        for k in range(RD):
            nc.gpsimd.memset(Dring[k][:, :], BIG)
        nc.gpsimd.memset(Hall[:, :], BIG)
        for q in range(NQ):
            for r in range(min(3, B - 3 * q)):
                nc.sync.dma_start(lq[q][32 * r + 1 : 32 * r + 2, :], ones_row[:, :])
                nc.gpsimd.memset(rq[q][32 * r : 32 * r + 1, :], 1.0)
        if sim_safe:
            nc.gpsimd.memset(sheared[:, :], 0.0)
        else:
            # ramp-up triangles only (consumed while block c waits for row 0)
            for c in range(1, NB):
                g = 32 * c
                nc.gpsimd.memset(sheared[g : g + B, 0 : STG * c * BS], 0.0)

        def load_transpose(src, dst_slice, scale, b):
            ld = lpool.tile([128, RC * D], f32, tag="ld", name="ld")
            # s2 loads on SP; s1 loads via Pool SWDGE (Act is the s1-pass
            # bottleneck, Pool has slack there)
            eng = nc.sync if scale == 1.0 else nc.gpsimd
            eng.dma_start(
                ld[:, :].rearrange("p (rc d) -> p rc d", rc=RC),
                src[b].rearrange("(rc p) d -> p rc d", p=128),
            )
            tp = tpool.tile([128, L], f32, tag="tp", name="tp")
            for rc in range(RC):
                nc.tensor.transpose(
                    tp[:, rc * 128 : (rc + 1) * 128],
                    ld[:, rc * D : (rc + 1) * D],
                    identf[:, :],
                )
            nc.scalar.activation(dst_slice, tp[:, :], Act.Copy, scale=scale)

        def cost_chunk(rc, b):
            q, r = b // 3, b % 3
            cp = cpool.tile([128, L], f32, tag="cp", name="cp")
            nc.tensor.matmul(
                cp[:, :],
                s1T[:, b * L + rc * 128 : b * L + (rc + 1) * 128],
                s2T[:, b * L : (b + 1) * L],
                start=True,
                stop=False,
            )
            nc.tensor.matmul(
                cp[:, :],
                lq[q][32 * r : 32 * r + 2, rc * 128 : (rc + 1) * 128],
                rq[q][32 * r : 32 * r + 2, :],
                start=False,
                stop=True,
            )
            csb = cspool.tile([128, L], bf16, tag="csb", name="csb")
            nc.scalar.activation(csb[:, :], cp[:, :], Act.Copy)
            for c in range(NB):
                deng = nc.sync if c % 2 == 0 else nc.scalar
                deng.dma_start(
                    sheared[
                        32 * c + b : 32 * c + b + 1,
                        (rc * 128 + STG * c) * BS : (rc * 128 + 128 + STG * c) * BS,
                    ],
                    csb[:, c * BS : (c + 1) * BS],
                )

        # --- phase A, s2 pass: transposes + n2 for all batches ---
        for b in range(B):
            q, r = b // 3, b % 3
            sl2 = s2T[:, b * L : (b + 1) * L]
            load_transpose(s2, sl2, 1.0, b)
            sq2 = sqpool.tile([128, L], bf16, tag="sq2", name="sq2")
            nc.gpsimd.tensor_tensor(out=sq2[:, :], in0=sl2, in1=sl2, op=Alu.mult)
            n2p = npool.tile([1, L], f32, tag="n2p", name="n2p")
            nc.tensor.matmul(n2p[:, :], ones[:, :], sq2[:, :], start=True, stop=True)
            n2sb = sqpool.tile([1, L], bf16, tag="n2sb", name="n2sb")
            nc.scalar.activation(n2sb[:, :], n2p[:, :], Act.Copy)
            nc.sync.dma_start(rq[q][32 * r + 1 : 32 * r + 2, :], n2sb[:, :])

        # --- phase A, s1 pass + rc-0 cost (starts the wavefront early) ---
        for b in range(B):
            q, r = b // 3, b % 3
            sl1 = s1T[:, b * L : (b + 1) * L]
            load_transpose(s1, sl1, -2.0, b)
            sq1 = sqpool.tile([128, L], bf16, tag="sq1", name="sq1")
            nc.gpsimd.tensor_tensor(out=sq1[:, :], in0=sl1, in1=sl1, op=Alu.mult)
            n1p = npool.tile([1, L], f32, tag="n1p", name="n1p")
            nc.tensor.matmul(n1p[:, :], q_ones[:, :], sq1[:, :], start=True, stop=True)
            nc.vector.tensor_copy(lq[q][32 * r : 32 * r + 1, :], n1p[:, :])
            cost_chunk(0, b)

        # --- phase A tail: row chunks 1..RC-1 (overlap the running DP) ---
        for rc in range(1, RC):
            for b in range(B):
                cost_chunk(rc, b)

        # --- phase B: staggered wavefront DP, one fused DVE op per step ---
        # After Act's phase-A tail drains (~step 180 of the wavefront), move
        # two of the three halo copies to Act; before that, Act is still
        # busy with rc1/rc2 cost work and the offload backfires.
        HSPLIT = 130

        def emit_halo(s):
            # Hr_{s+2}[32c+b] <- r of step s, block c-1, last column. Serves
            # scan_{s+2} as r[-1] (s0) and scan_{s+3} as p[-1] (s1).
            Ha = Dring[s % RD]
            for g in range(1, NB):
                dst = Hall[32 * g : 32 * g + 32, s + 2 : s + 3]
                src = Ha[32 * (g - 1) : 32 * (g - 1) + 32, BS - 1 : BS]
                if s >= HSPLIT and g >= 2:
                    nc.scalar.activation(dst, src, Act.Copy)
                else:
                    nc.gpsimd.tensor_copy(dst, src)

        nc.vector._custom_dve(
            fused,
            out=Dring[0][0:B, :],
            in0=Dring[RD - 1][0:B, :],  # all BIG -> out = cumsum
            in1=sheared[0:B, 0:BS],
            s0=0.0,
            s1=0.0,
        )
        for s in range(1, NSTEP):
            nc.vector._custom_dve(
                fused,
                out=Dring[s % RD][:, :],
                in0=Dring[(s - 1) % RD][:, :],
                in1=sheared[:, s * BS : (s + 1) * BS],
                s0=Hall[:, s : s + 1],
                s1=Hall[:, s - 1 : s],
            )
            if s + 2 < NSTEP:
                emit_halo(s)

        Dlast = Dring[(NSTEP - 1) % RD]
        gl = 32 * (NB - 1)
        nc.sync.dma_start(out, Dlast[gl : gl + B, BS - 1 : BS])

    nc.finalize()
    return nc


_IDENT = np.eye(128, dtype=np.float32)


def kernel(s1_batch: np.ndarray, s2_batch: np.ndarray) -> np.ndarray:
    _ensure_path()
    from concourse.bass_utils import run_bass_kernel_spmd

    s1 = np.ascontiguousarray(s1_batch, dtype=np.float32)
    s2 = np.ascontiguousarray(s2_batch, dtype=np.float32)
    Btot, L, D = s1.shape
    assert Btot % N_CORES == 0
    B = Btot // N_CORES

    nc = build_nc(B, L, D, NB=4)
    in_maps = [
        {
            "s1": s1[i * B : (i + 1) * B],
            "s2": s2[i * B : (i + 1) * B],
            "ident": _IDENT,
        }
        for i in range(N_CORES)
    ]
    res = run_bass_kernel_spmd(nc, in_maps, list(range(N_CORES)))
    finals = np.concatenate([r["dtw"] for r in res.results])
    return np.array(np.mean(finals.astype(np.float64)), dtype=np.float32)
